# revision 37
# baseline (speedup 1.0000x reference)
"""Trainium2 Bass kernel for the SNN Leaky-Integrate-Fire problem.

Layout per core (8192 rows): partition p = r4*32 + h (r4 = row-group 0..3,
h = hidden 0..31); free index j in [0, 2048); logical row r = r4*2048 + j.
This puts the hidden dim in the partition axis, so fc1/fc2 become PE
matmuls over the partition (k) dim and all h-dependent constants are
per-partition scalars:

  PE:  cn = -(x @ W1.T + b1)  via stationary S1[13,128] (-W1/-b1 baked in,
       ones row for the bias), moving xj[13,2048] -> PSUM [128,2048] fp32
  ACT: copy cn PSUM -> SBUF
  DVE: 100-step LIF scan on negated state n = -mem, n' = beta*n+cn+(n<-1),
       2 steps fused per custom-DVE instruction (50 instructions), spike
       emitted as fp16
  PE:  out = spk @ W2.T  via stationary S2[128,12] -> PSUM [12,2048]
  ACT: + b2 and evict to SBUF, DMA out

The custom DVE ops are registered at import time (runtime-patch of
concourse.dve_ops.OPS); their per-stage fp32 rounding matches the
reference's fl(fl(fl(beta*m)+cur1)-h) sequence bit-for-bit.
"""
import sys

sys.path.insert(0, "/opt/trn_rl_repo")

import numpy as np

import concourse.bacc as bacc
import concourse.tile as tile
from concourse import mybir
from concourse import dve_ops as dvo
from concourse.dve_spec import Spec, Src0, Src1, C0, C1, lower, _has_src1
from concourse.dve_uop import DveOpSpec
from concourse.bass_utils import run_bass_kernel_spmd

F32 = mybir.dt.float32
F16 = mybir.dt.float16
ALU = mybir.AluOpType
AF = mybir.ActivationFunctionType

# problem constants (hardcoded per contract)
B, N_IN, N_HID, N_OUT = 65536, 3, 32, 3
NUM_STEPS, BETA, THR = 100, 0.9, 1.0
N_CORES = 8
BC = B // N_CORES          # rows per core = 8192
P = 128                    # partitions
NG = 4                     # row groups per core
J = BC // NG               # rows per group = free size = 2048
KM = N_IN * NG + 1         # moving rows for fc1 = 13 (x features x groups + ones)
QO = N_OUT * NG            # fc2 output partitions = 12


# --- custom DVE op registration (runtime-patch of dve_ops.OPS) -------------


def _nstep(n, cn, b, th):
    return ((n * np.float32(b) + cn).astype(np.float32) + (n < th)).astype(
        np.float32
    )


def _ref_lif2_b0(in0, in1, s0, s1, imm2):
    n = in0.astype(np.float32)
    return _nstep(_nstep(n, n, s0, s1), n, s0, s1)


def _ref_lif2(in0, in1, s0, s1, imm2):
    cn = in1.astype(np.float32)
    return _nstep(_nstep(in0.astype(np.float32), cn, s0, s1), cn, s0, s1)


def _ref_lif1_spk(in0, in1, s0, s1, imm2):
    n = _nstep(in0.astype(np.float32), in1.astype(np.float32), s0, s1)
    return (n < np.float32(s1)).astype(np.float32)


def _register_op(name, spec):
    for o in dvo.OPS:
        if o.name == name:
            return o
    row = dvo._CUSTOM_DVE_ROW_BASE + len(dvo.OPS)
    dvo._SUB_OPCODE_FOR_NAME[name] = row
    uops = lower(spec, ver="v3")
    sha = DveOpSpec(name=name, opcode=row, uops=uops, rd1_en=_has_src1(spec)).sha(
        "v3"
    )
    op = dvo.DveOp(name, spec, subdim=False, uops_sha={"v3": sha})
    dvo.OPS.append(op)
    dvo.CUSTOM_DVE_SPECS[name] = spec
    return op


def _make_ops():
    n2_ = (Src0 * C0 + Src0) + (Src0 < C1)
    n3_ = (n2_ * C0 + Src0) + (n2_ < C1)
    b0 = _register_op("LIF2_B0_ANT", Spec(body=n3_, reference=_ref_lif2_b0))
    s1_ = (Src0 * C0 + Src1) + (Src0 < C1)
    s2_ = (s1_ * C0 + Src1) + (s1_ < C1)
    l2 = _register_op("LIF2_ANT", Spec(body=s2_, reference=_ref_lif2))
    spk = _register_op(
        "LIF1_SPK_ANT", Spec(body=(s1_ < C1), reference=_ref_lif1_spk)
    )
    return b0, l2, spk


LIF2_B0, LIF2, LIF1_SPK = _make_ops()


KB = 38              # fp16-split fc1 contraction rows: [xh,1 | xh | xl,1]


def build(nc, num_steps=NUM_STEPS):
    s1_d = nc.dram_tensor("s1", [KB, P], F16, kind="ExternalInput")
    xa_d = nc.dram_tensor("xa", [KB, J], F16, kind="ExternalInput")
    s2_d = nc.dram_tensor("s2", [P, QO + 1], F16, kind="ExternalInput")
    y_d = nc.dram_tensor("y", [QO, J], F16, kind="ExternalOutput")

    dve = nc.vector

    with tile.TileContext(nc) as tc:
        with tc.tile_pool(name="pool", bufs=1) as pool, \
             tc.tile_pool(name="ps", bufs=1, space="PSUM") as psp:
            # s1 first (unblocks LDWEIGHTS), then x in 4 chunk DMAs so the
            # first fc1 matmuls start before the full transfer lands
            # (only the SP HWDGE queue works under this runtime)
            s1tt = pool.tile([KB, P], F16, tag="s1t")
            nc.sync.dma_start(s1tt[:], s1_d[:], single_packet=True)
            xat = pool.tile([KB, J], F16, tag="xat")
            NB = 4  # 512-col PSUM banks
            for b in range(NB):
                sl = slice(b * (J // NB), (b + 1) * (J // NB))
                nc.sync.dma_start(xat[:, sl], xa_d[:, sl], single_packet=True)
            s2t = pool.tile([P, QO + 1], F16, tag="s2t")
            nc.sync.dma_start(s2t[:], s2_d[:], single_packet=True)
            xjt = xat[:]
            s1t = s1tt[:]
            b2t = s2t[:QO, QO : QO + 1]

            cnp = psp.tile([P, J], F32, tag="cnp")
            cn = pool.tile([P, J], F32, tag="cn")
            na = pool.tile([P, J], F32, tag="na")
            nb = pool.tile([P, J], F32, tag="nb")
            spkb = pool.tile([P, J], F16, tag="spkb")
            yp = psp.tile([QO, J], F32, tag="yp")
            yt = pool.tile([QO, J], F16, tag="yt")

            # --- fc1 on PE: cn = -(x @ W1.T + b1) into PSUM ---
            for b in range(NB):
                sl = slice(b * (J // NB), (b + 1) * (J // NB))
                nc.tensor.matmul(
                    cnp[:, sl], s1t, xjt[:, sl], start=True, stop=True
                )
                if b % 2 == 1:
                    # ACT copies each finished half PSUM -> SBUF while the
                    # DVE's seed op (reading PSUM directly) already runs
                    hsl = slice((b - 1) * (J // NB), (b + 1) * (J // NB))
                    nc.scalar.copy(cn[:, hsl], cnp[:, hsl])

            # --- scan: steps 2..3 seeded from n1 = cn, then 2 steps/instr.
            # Two independent half-column chains interleaved to hide the
            # dependent-write ack latency. Half 1 leads by 2 instructions so
            # it starts as soon as its half of cn is copied, while half 2's
            # matmuls/copy still run.
            H = J // 2
            halves = [(cn[:, :H], na[:, :H], nb[:, :H]),
                      (cn[:, H:], na[:, H:], nb[:, H:])]
            states = [[a_, b_] for c_, a_, b_ in halves]
            n_lif2 = (num_steps - 4) // 2  # 48 LIF2 instructions per half
            LEAD = 2

            def emit_b0(hi):
                c_, a_, b_ = halves[hi]
                dve._custom_dve(LIF2_B0, out=a_, in0=c_, s0=BETA, s1=-THR)

            def emit_l2(hi):
                c_ = halves[hi][0]
                cur_, nxt_ = states[hi]
                dve._custom_dve(
                    LIF2, out=nxt_, in0=cur_, in1=c_, s0=BETA, s1=-THR
                )
                states[hi] = [nxt_, cur_]

            def emit_spk(hi):
                c_ = halves[hi][0]
                cur_, nxt_ = states[hi]
                dve._custom_dve(
                    LIF1_SPK, out=spkb[:, hi * H : (hi + 1) * H], in0=cur_,
                    in1=c_, s0=BETA, s1=-THR,
                )

            emit_b0(0)
            for _ in range(LEAD):
                emit_l2(0)
            emit_b0(1)
            for _ in range(n_lif2 - LEAD):
                emit_l2(1)
                emit_l2(0)
            emit_spk(0)
            for _ in range(LEAD):
                emit_l2(1)
            emit_spk(1)

            # --- fc2 on PE: yp[(o,r4), j] = sum_h W2[o,h] spk[(r4,h), j] ---
            with nc.allow_low_precision(reason="fc2 spk/W2 in fp16, fp32 accum"):
                for b in range(NB):
                    sl = slice(b * (J // NB), (b + 1) * (J // NB))
                    nc.tensor.matmul(
                        yp[:, sl], s2t[:, :QO], spkb[:, sl], start=True,
                        stop=True,
                    )
                    if b % 2 == 1:
                        # ACT: + b2 and evict PSUM -> SBUF, then DMA, per half
                        hsl = slice((b - 1) * (J // NB), (b + 1) * (J // NB))
                        nc.scalar.activation(
                            yt[:, hsl], yp[:, hsl], AF.Identity, bias=b2t,
                            scale=1.0,
                        )
                        nc.sync.dma_start(y_d[:, hsl], yt[:, hsl])
    return nc


_CACHE = {}


def _get_program():
    if "nc" not in _CACHE:
        nc = bacc.Bacc("TRN2", target_bir_lowering=False, debug=False,
                       num_devices=N_CORES)
        build(nc)
        nc.compile()
        _CACHE["nc"] = nc
    return _CACHE["nc"]


def _make_consts(W1, b1, W2, b2):
    bf = np.float16
    W1h = W1.astype(bf).astype(np.float32)
    W1l = (W1 - W1h).astype(bf).astype(np.float32)
    b1h = b1.astype(bf).astype(np.float32)
    b1l = (b1 - b1h).astype(bf).astype(np.float32)
    # stationary [KB, P]: rows 0-12: [-W1h | -b1h]; 13-24: [-W1l];
    # rows 25-37: [-W1h | -b1l] (pairs with [xh,1 | xh | xl,1] moving)
    s1 = np.zeros((KB, P), dtype=np.float32)
    for i in range(N_IN):
        for r4 in range(NG):
            cols = slice(r4 * N_HID, (r4 + 1) * N_HID)
            s1[i * NG + r4, cols] = -W1h[:, i]
            s1[13 + i * NG + r4, cols] = -W1l[:, i]
            s1[25 + i * NG + r4, cols] = -W1h[:, i]
    for r4 in range(NG):
        cols = slice(r4 * N_HID, (r4 + 1) * N_HID)
        s1[12, cols] = -b1h
        s1[KB - 1, cols] = -b1l
    s2 = np.zeros((P, QO + 1), dtype=np.float16)
    for o in range(N_OUT):
        for r4 in range(NG):
            s2[r4 * N_HID : (r4 + 1) * N_HID, o * NG + r4] = W2[o].astype(
                np.float16
            )
            s2[o * NG + r4, QO] = np.float16(b2[o])
    return s1.astype(bf), s2


def kernel(x, W1, b1, W2, b2):
    x = np.asarray(x, dtype=np.float32)
    W1, b1, W2, b2 = (np.asarray(a, dtype=np.float32) for a in (W1, b1, W2, b2))
    bf = np.float16
    s1, s2 = _make_consts(W1, b1, W2, b2)
    nc = _get_program()
    xh = x.astype(bf)
    xl = (x - xh.astype(np.float32)).astype(bf)
    in_maps = []
    for i in range(N_CORES):
        sl = slice(i * BC, (i + 1) * BC)
        xhc = xh[sl].reshape(NG, J, N_IN).transpose(2, 0, 1).reshape(12, J)
        xlc = xl[sl].reshape(NG, J, N_IN).transpose(2, 0, 1).reshape(12, J)
        xa = np.empty((KB, J), dtype=bf)
        xa[:12] = xhc
        xa[12] = bf(1.0)
        xa[13:25] = xhc
        xa[25:37] = xlc
        xa[37] = bf(1.0)
        in_maps.append({"s1": s1, "xa": xa, "s2": s2})
    kwargs = dict(_CACHE.get("run_kwargs") or {})
    res = run_bass_kernel_spmd(nc, in_maps, core_ids=list(range(N_CORES)), **kwargs)
    _CACHE["last_results"] = res
    # y[(o*NG+r4), j] <-> out[r4*J + j, o]
    out = np.empty((B, N_OUT), dtype=np.float32)
    for i in range(N_CORES):
        yc = res.results[i]["y"].reshape(N_OUT, NG, J)
        out[i * BC : (i + 1) * BC] = yc.transpose(1, 2, 0).reshape(BC, N_OUT)
    return out


# revision 38
# speedup vs baseline: 1.1908x; 1.1908x over previous
"""Trainium2 Bass kernel for the SNN Leaky-Integrate-Fire problem.

Layout per core (8192 rows): partition p = r4*32 + h (r4 = row-group 0..3,
h = hidden 0..31); free index j in [0, 2048); logical row r = r4*2048 + j.
This puts the hidden dim in the partition axis, so fc1/fc2 become PE
matmuls over the partition (k) dim and all h-dependent constants are
per-partition scalars:

  PE:  cn = -(x @ W1.T + b1)  via stationary S1[13,128] (-W1/-b1 baked in,
       ones row for the bias), moving xj[13,2048] -> PSUM [128,2048] fp32
  ACT: copy cn PSUM -> SBUF
  DVE: 100-step LIF scan on negated state n = -mem, n' = beta*n+cn+(n<-1),
       2 steps fused per custom-DVE instruction (50 instructions), spike
       emitted as fp16
  PE:  out = spk @ W2.T  via stationary S2[128,12] -> PSUM [12,2048]
  ACT: + b2 and evict to SBUF, DMA out

The custom DVE ops are registered at import time (runtime-patch of
concourse.dve_ops.OPS); their per-stage fp32 rounding matches the
reference's fl(fl(fl(beta*m)+cur1)-h) sequence bit-for-bit.
"""
import sys

sys.path.insert(0, "/opt/trn_rl_repo")

import numpy as np

import concourse.bacc as bacc
import concourse.tile as tile
from concourse import mybir
from concourse import dve_ops as dvo
from concourse.dve_spec import Spec, Src0, Src1, C0, C1, lower, _has_src1
from concourse.dve_uop import DveOpSpec
from concourse.bass_utils import run_bass_kernel_spmd

F32 = mybir.dt.float32
F16 = mybir.dt.float16
ALU = mybir.AluOpType
AF = mybir.ActivationFunctionType

# problem constants (hardcoded per contract)
B, N_IN, N_HID, N_OUT = 65536, 3, 32, 3
NUM_STEPS, BETA, THR = 100, 0.9, 1.0
N_CORES = 8
BC = B // N_CORES          # rows per core = 8192
P = 128                    # partitions
NG = 4                     # row groups per core
J = BC // NG               # rows per group = free size = 2048
KM = N_IN * NG + 1         # moving rows for fc1 = 13 (x features x groups + ones)
QO = N_OUT * NG            # fc2 output partitions = 12


# --- custom DVE op registration (runtime-patch of dve_ops.OPS) -------------


def _nstep(n, cn, b, th):
    return ((n * np.float32(b) + cn).astype(np.float32) + (n < th)).astype(
        np.float32
    )


def _ref_lif2_b0(in0, in1, s0, s1, imm2):
    n = in0.astype(np.float32)
    return _nstep(_nstep(n, n, s0, s1), n, s0, s1)


def _ref_lif2(in0, in1, s0, s1, imm2):
    cn = in1.astype(np.float32)
    return _nstep(_nstep(in0.astype(np.float32), cn, s0, s1), cn, s0, s1)


def _ref_lif1_spk(in0, in1, s0, s1, imm2):
    n = _nstep(in0.astype(np.float32), in1.astype(np.float32), s0, s1)
    return (n < np.float32(s1)).astype(np.float32)


def _register_op(name, spec):
    for o in dvo.OPS:
        if o.name == name:
            return o
    row = dvo._CUSTOM_DVE_ROW_BASE + len(dvo.OPS)
    dvo._SUB_OPCODE_FOR_NAME[name] = row
    uops = lower(spec, ver="v3")
    sha = DveOpSpec(name=name, opcode=row, uops=uops, rd1_en=_has_src1(spec)).sha(
        "v3"
    )
    op = dvo.DveOp(name, spec, subdim=False, uops_sha={"v3": sha})
    dvo.OPS.append(op)
    dvo.CUSTOM_DVE_SPECS[name] = spec
    return op


def _make_ops():
    n2_ = (Src0 * C0 + Src0) + (Src0 < C1)
    n3_ = (n2_ * C0 + Src0) + (n2_ < C1)
    b0 = _register_op("LIF2_B0_ANT", Spec(body=n3_, reference=_ref_lif2_b0))
    s1_ = (Src0 * C0 + Src1) + (Src0 < C1)
    s2_ = (s1_ * C0 + Src1) + (s1_ < C1)
    l2 = _register_op("LIF2_ANT", Spec(body=s2_, reference=_ref_lif2))
    spk = _register_op(
        "LIF1_SPK_ANT", Spec(body=(s1_ < C1), reference=_ref_lif1_spk)
    )
    return b0, l2, spk


LIF2_B0, LIF2, LIF1_SPK = _make_ops()


KB = 38              # fp16-split fc1 contraction rows: [xh,1 | xh | xl,1]


def build(nc, num_steps=NUM_STEPS):
    s1_d = nc.dram_tensor("s1", [KB, P], F16, kind="ExternalInput")
    xa_d = nc.dram_tensor("xa", [KB, J], F16, kind="ExternalInput")
    s2_d = nc.dram_tensor("s2", [P, QO + 1], F16, kind="ExternalInput")
    y_d = nc.dram_tensor("y", [QO, J], F32, kind="ExternalOutput")

    dve = nc.vector

    with tile.TileContext(nc) as tc:
        with tc.tile_pool(name="pool", bufs=1) as pool, \
             tc.tile_pool(name="ps", bufs=1, space="PSUM") as psp:
            # s1 first (unblocks LDWEIGHTS), then x in 4 chunk DMAs so the
            # first fc1 matmuls start before the full transfer lands
            # (only the SP HWDGE queue works under this runtime)
            s1tt = pool.tile([KB, P], F16, tag="s1t")
            nc.sync.dma_start(s1tt[:], s1_d[:], single_packet=True)
            xat = pool.tile([KB, J], F16, tag="xat")
            NB = 4  # 512-col PSUM banks
            for b in range(NB):
                sl = slice(b * (J // NB), (b + 1) * (J // NB))
                nc.sync.dma_start(xat[:, sl], xa_d[:, sl], single_packet=True)
            s2t = pool.tile([P, QO + 1], F16, tag="s2t")
            nc.sync.dma_start(s2t[:], s2_d[:], single_packet=True)
            xjt = xat[:]
            s1t = s1tt[:]
            b2t = s2t[:QO, QO : QO + 1]

            cnp = psp.tile([P, J], F32, tag="cnp")
            cn = pool.tile([P, J], F32, tag="cn")
            na = pool.tile([P, J], F32, tag="na")
            nb = pool.tile([P, J], F32, tag="nb")
            spkb = pool.tile([P, J], F16, tag="spkb")
            yp = psp.tile([QO, J], F32, tag="yp")
            yt = pool.tile([QO, J], F32, tag="yt")

            # --- fc1 on PE: cn = -(x @ W1.T + b1) into PSUM ---
            for b in range(NB):
                sl = slice(b * (J // NB), (b + 1) * (J // NB))
                nc.tensor.matmul(
                    cnp[:, sl], s1t, xjt[:, sl], start=True, stop=True
                )
                if b % 2 == 1:
                    # ACT copies each finished half PSUM -> SBUF while the
                    # DVE's seed op (reading PSUM directly) already runs
                    hsl = slice((b - 1) * (J // NB), (b + 1) * (J // NB))
                    nc.scalar.copy(cn[:, hsl], cnp[:, hsl])

            # --- scan: steps 2..3 seeded from n1 = cn, then 2 steps/instr.
            # Two independent half-column chains interleaved to hide the
            # dependent-write ack latency. Half 1 leads by 2 instructions so
            # it starts as soon as its half of cn is copied, while half 2's
            # matmuls/copy still run.
            H = J // 2
            halves = [(cn[:, :H], na[:, :H], nb[:, :H]),
                      (cn[:, H:], na[:, H:], nb[:, H:])]
            states = [[a_, b_] for c_, a_, b_ in halves]
            n_lif2 = (num_steps - 4) // 2  # 48 LIF2 instructions per half
            LEAD = 2

            def emit_b0(hi):
                c_, a_, b_ = halves[hi]
                dve._custom_dve(LIF2_B0, out=a_, in0=c_, s0=BETA, s1=-THR)

            def emit_l2(hi):
                c_ = halves[hi][0]
                cur_, nxt_ = states[hi]
                dve._custom_dve(
                    LIF2, out=nxt_, in0=cur_, in1=c_, s0=BETA, s1=-THR
                )
                states[hi] = [nxt_, cur_]

            def emit_spk(hi):
                c_ = halves[hi][0]
                cur_, nxt_ = states[hi]
                dve._custom_dve(
                    LIF1_SPK, out=spkb[:, hi * H : (hi + 1) * H], in0=cur_,
                    in1=c_, s0=BETA, s1=-THR,
                )

            emit_b0(0)
            for _ in range(LEAD):
                emit_l2(0)
            emit_b0(1)
            for _ in range(n_lif2 - LEAD):
                emit_l2(1)
                emit_l2(0)
            emit_spk(0)
            for _ in range(LEAD):
                emit_l2(1)
            emit_spk(1)

            # --- fc2 on PE: yp[(o,r4), j] = sum_h W2[o,h] spk[(r4,h), j] ---
            with nc.allow_low_precision(reason="fc2 spk/W2 in fp16, fp32 accum"):
                for b in range(NB):
                    sl = slice(b * (J // NB), (b + 1) * (J // NB))
                    nc.tensor.matmul(
                        yp[:, sl], s2t[:, :QO], spkb[:, sl], start=True,
                        stop=True,
                    )
                    if b % 2 == 1:
                        # ACT: + b2 and evict PSUM -> SBUF, then DMA, per half
                        hsl = slice((b - 1) * (J // NB), (b + 1) * (J // NB))
                        nc.scalar.activation(
                            yt[:, hsl], yp[:, hsl], AF.Identity, bias=b2t,
                            scale=1.0,
                        )
                        nc.sync.dma_start(y_d[:, hsl], yt[:, hsl])
    return nc


_CACHE = {}


def _get_program():
    if "nc" not in _CACHE:
        nc = bacc.Bacc("TRN2", target_bir_lowering=False, debug=False,
                       num_devices=N_CORES)
        build(nc)
        nc.compile()
        _CACHE["nc"] = nc
    return _CACHE["nc"]


def _make_consts(W1, b1, W2, b2):
    bf = np.float16
    W1h = W1.astype(bf).astype(np.float32)
    W1l = (W1 - W1h).astype(bf).astype(np.float32)
    b1h = b1.astype(bf).astype(np.float32)
    b1l = (b1 - b1h).astype(bf).astype(np.float32)
    # stationary [KB, P]: rows 0-12: [-W1h | -b1h]; 13-24: [-W1l];
    # rows 25-37: [-W1h | -b1l] (pairs with [xh,1 | xh | xl,1] moving)
    s1 = np.zeros((KB, P), dtype=np.float32)
    for i in range(N_IN):
        for r4 in range(NG):
            cols = slice(r4 * N_HID, (r4 + 1) * N_HID)
            s1[i * NG + r4, cols] = -W1h[:, i]
            s1[13 + i * NG + r4, cols] = -W1l[:, i]
            s1[25 + i * NG + r4, cols] = -W1h[:, i]
    for r4 in range(NG):
        cols = slice(r4 * N_HID, (r4 + 1) * N_HID)
        s1[12, cols] = -b1h
        s1[KB - 1, cols] = -b1l
    s2 = np.zeros((P, QO + 1), dtype=np.float16)
    for o in range(N_OUT):
        for r4 in range(NG):
            s2[r4 * N_HID : (r4 + 1) * N_HID, o * NG + r4] = W2[o].astype(
                np.float16
            )
            s2[o * NG + r4, QO] = np.float16(b2[o])
    return s1.astype(bf), s2


def kernel(x, W1, b1, W2, b2):
    x = np.asarray(x, dtype=np.float32)
    W1, b1, W2, b2 = (np.asarray(a, dtype=np.float32) for a in (W1, b1, W2, b2))
    bf = np.float16
    s1, s2 = _make_consts(W1, b1, W2, b2)
    nc = _get_program()
    xh = x.astype(bf)
    xl = (x - xh.astype(np.float32)).astype(bf)
    in_maps = []
    for i in range(N_CORES):
        sl = slice(i * BC, (i + 1) * BC)
        xhc = xh[sl].reshape(NG, J, N_IN).transpose(2, 0, 1).reshape(12, J)
        xlc = xl[sl].reshape(NG, J, N_IN).transpose(2, 0, 1).reshape(12, J)
        xa = np.empty((KB, J), dtype=bf)
        xa[:12] = xhc
        xa[12] = bf(1.0)
        xa[13:25] = xhc
        xa[25:37] = xlc
        xa[37] = bf(1.0)
        in_maps.append({"s1": s1, "xa": xa, "s2": s2})
    kwargs = dict(_CACHE.get("run_kwargs") or {})
    res = run_bass_kernel_spmd(nc, in_maps, core_ids=list(range(N_CORES)), **kwargs)
    _CACHE["last_results"] = res
    # y[(o*NG+r4), j] <-> out[r4*J + j, o]
    out = np.empty((B, N_OUT), dtype=np.float32)
    for i in range(N_CORES):
        yc = res.results[i]["y"].reshape(N_OUT, NG, J)
        out[i * BC : (i + 1) * BC] = yc.transpose(1, 2, 0).reshape(BC, N_OUT)
    return out
